# revision 10
# baseline (speedup 1.0000x reference)
"""PointMultiGraspNet-V3 segment_reduce kernel for 8 Trainium2 NeuronCores.

Strategy: channel-parallel segment-max.  feat is transposed on host to
(C=1024, N=65536); core d owns channels [128d, 128d+128) for ALL points, so
the per-segment reduce ranges (derived from `offsets` at build time) are
identical on every core and the SPMD program is fully static.  Each core
streams its (128, N) slice from HBM and does per-segment reduce_max along the
free dim -- the kernel is HBM-bandwidth bound (32 MiB/core).  The tiny MLP
head needs all 1024 channels, so each core computes its partial
features @ W1.T (contraction over its own 128 channels), the partials are
AllReduce-summed (256x512 = 512 KB), and the rest of the MLP is replicated on
every core.  `features` itself is assembled on host from the per-core channel
slices; pred/offset are taken from core 0.
"""

import math
from contextlib import ExitStack

import numpy as np

import concourse.bacc as bacc
import concourse.bass as bass
import concourse.tile as tile
from concourse import mybir
from concourse.bass_utils import run_bass_kernel_spmd
from concourse.masks import make_identity

NCORES = 8
P = 128            # SBUF partitions == channels per core
EPS = 1e-5
KPAD = 512         # padded slots per segment in the original model
NEG = -3.0e38      # -inf stand-in (finite so 0*x etc. stay finite)
FP = mybir.dt.float32
AXX = mybir.AxisListType.X


def _build_program(bounds, n_points, cw):
    """Build the SPMD Bass program.

    bounds: sequence of B+1 ints, bounds[0] == 0, bounds[-1] == n_points;
            segment s covers points [bounds[s], bounds[s+1]).
    n_points: total points (free-dim length of the per-core featT slice).
    cw: streaming chunk width in points.
    """
    B = len(bounds) - 1
    SB = (B + 127) // 128          # segment blocks of 128 (2 for B=256)
    assert B % 128 == 0, "segment count must be a multiple of 128"

    nc = bacc.Bacc("TRN2", target_bir_lowering=False, debug=False,
                   num_devices=NCORES)

    # ---- DRAM I/O -------------------------------------------------------
    featT = nc.dram_tensor("featT", [P, n_points], FP, kind="ExternalInput")
    floor_d = nc.dram_tensor("floor", [1, B], FP, kind="ExternalInput")
    w1t_d = nc.dram_tensor("w1t", [P, 512], FP, kind="ExternalInput")
    b1_d = nc.dram_tensor("b1", [1, 512], FP, kind="ExternalInput")
    ln1g_d = nc.dram_tensor("ln1g", [1, 512], FP, kind="ExternalInput")
    ln1b_d = nc.dram_tensor("ln1b", [1, 512], FP, kind="ExternalInput")
    infoT_d = nc.dram_tensor("infoT", [4, B], FP, kind="ExternalInput")
    wit_d = nc.dram_tensor("wit", [4, 32], FP, kind="ExternalInput")
    bi_d = nc.dram_tensor("bi", [1, 32], FP, kind="ExternalInput")
    wa1t_d = nc.dram_tensor("wa1t", [544, 256], FP, kind="ExternalInput")
    ba1_d = nc.dram_tensor("ba1", [1, 256], FP, kind="ExternalInput")
    lnag_d = nc.dram_tensor("lnag", [1, 256], FP, kind="ExternalInput")
    lnab_d = nc.dram_tensor("lnab", [1, 256], FP, kind="ExternalInput")
    wa2t_d = nc.dram_tensor("wa2t", [256, 6], FP, kind="ExternalInput")
    ba2_d = nc.dram_tensor("ba2", [1, 6], FP, kind="ExternalInput")
    wo1t_d = nc.dram_tensor("wo1t", [544, 256], FP, kind="ExternalInput")
    bo1_d = nc.dram_tensor("bo1", [1, 256], FP, kind="ExternalInput")
    lnog_d = nc.dram_tensor("lnog", [1, 256], FP, kind="ExternalInput")
    lnob_d = nc.dram_tensor("lnob", [1, 256], FP, kind="ExternalInput")
    wo2t_d = nc.dram_tensor("wo2t", [256, 18], FP, kind="ExternalInput")
    bo2_d = nc.dram_tensor("bo2", [1, 18], FP, kind="ExternalInput")

    featTout_d = nc.dram_tensor("featT_out", [P, B], FP, kind="ExternalOutput")
    pred_d = nc.dram_tensor("pred_out", [B, 6], FP, kind="ExternalOutput")
    off_d = nc.dram_tensor("off_out", [B, 18], FP, kind="ExternalOutput")

    nchunks = (n_points + cw - 1) // cw

    with ExitStack() as ctx:
        tc = ctx.enter_context(tile.TileContext(nc))
        singles = ctx.enter_context(tc.tile_pool(name="singles", bufs=1))
        chunks = ctx.enter_context(tc.tile_pool(name="chunks", bufs=3))
        small = ctx.enter_context(tc.tile_pool(name="small", bufs=4))
        psum = ctx.enter_context(tc.tile_pool(name="psum", bufs=1, space="PSUM"))
        dram = ctx.enter_context(tc.tile_pool(name="dram", bufs=1, space="DRAM"))

        def bcast(name, dram_t, n):
            t = singles.tile([P, n], FP, tag=name)
            nc.sync.dma_start(out=t[:], in_=dram_t[:].to_broadcast([P, n]))
            return t

        # ---- segment max over the streamed point dim --------------------
        acc = singles.tile([P, B], FP, tag="acc")
        nc.vector.memset(acc[:], NEG)

        for ci in range(nchunks):
            c0 = ci * cw
            c1 = min(n_points, c0 + cw)
            w = c1 - c0
            t = chunks.tile([P, cw], FP, tag="chunk")
            eng = nc.sync if ci % 2 == 0 else nc.scalar
            eng.dma_start(out=t[:, :w], in_=featT[:, c0:c1])
            for s in range(B):
                a = max(bounds[s], c0)
                b = min(bounds[s + 1], c1)
                if b <= a:
                    continue
                la, lb = a - c0, b - c0
                if bounds[s] >= c0:
                    # first touch of this segment: plain overwrite
                    nc.vector.reduce_max(out=acc[:, s:s + 1], in_=t[:, la:lb],
                                         axis=AXX)
                else:
                    tmp = small.tile([P, 1], FP, tag="tmp")
                    nc.vector.reduce_max(out=tmp[:], in_=t[:, la:lb], axis=AXX)
                    nc.vector.tensor_max(out=acc[:, s:s + 1],
                                         in0=acc[:, s:s + 1], in1=tmp[:])

        # floor: 0.0 where len < KPAD (relu clamp + empty segs), NEG otherwise
        fl = bcast("fl", floor_d, B)
        fT = singles.tile([P, B], FP, tag="fT")
        nc.vector.tensor_max(out=fT[:], in0=acc[:], in1=fl[:])
        nc.sync.dma_start(out=featTout_d[:, :], in_=fT[:])

        # ---- MLP head ---------------------------------------------------
        # partial features @ W1.T over this core's 128 channels
        w1sb = singles.tile([P, 512], FP, tag="w1sb")
        nc.sync.dma_start(out=w1sb[:], in_=w1t_d[:, :])
        ar_in = dram.tile([B, 512], FP, tag="ar_in")
        ar_out = dram.tile([B, 512], FP, tag="ar_out")
        for sb in range(SB):
            p1 = psum.tile([128, 512], FP, tag="p1")
            nc.tensor.matmul(p1[:], lhsT=fT[:, sb * 128:(sb + 1) * 128],
                             rhs=w1sb[:], start=True, stop=True)
            cp = small.tile([128, 512], FP, tag="cp")
            nc.scalar.activation(out=cp[:], in_=p1[:],
                                 func=mybir.ActivationFunctionType.Copy)
            nc.sync.dma_start(out=ar_in[sb * 128:(sb + 1) * 128, :], in_=cp[:])
        nc.gpsimd.collective_compute(
            "AllReduce", mybir.AluOpType.add,
            replica_groups=[list(range(NCORES))],
            ins=[ar_in[:].opt()], outs=[ar_out[:].opt()])

        # broadcast biases / LN params (persistent tiles)
        b1b = bcast("b1b", b1_d, 512)
        g1b = bcast("g1b", ln1g_d, 512)
        be1b = bcast("be1b", ln1b_d, 512)
        bib = bcast("bib", bi_d, 32)
        ba1b = bcast("ba1b", ba1_d, 256)
        gab = bcast("gab", lnag_d, 256)
        beab = bcast("beab", lnab_d, 256)
        ba2b = bcast("ba2b", ba2_d, 6)
        bo1b = bcast("bo1b", bo1_d, 256)
        gob = bcast("gob", lnog_d, 256)
        beob = bcast("beob", lnob_d, 256)
        bo2b = bcast("bo2b", bo2_d, 18)

        # weights
        infoT = singles.tile([4, B], FP, tag="infoT")
        nc.sync.dma_start(out=infoT[:], in_=infoT_d[:, :])
        wit = singles.tile([4, 32], FP, tag="wit")
        nc.sync.dma_start(out=wit[:], in_=wit_d[:, :])
        wa1s, wo1s = [], []
        for c in range(5):
            w = 128 if c < 4 else 32
            ta = singles.tile([w, 256], FP, tag=f"wa1_{c}")
            nc.sync.dma_start(out=ta[:], in_=wa1t_d[c * 128:c * 128 + w, :])
            wa1s.append(ta)
            to = singles.tile([w, 256], FP, tag=f"wo1_{c}")
            nc.sync.dma_start(out=to[:], in_=wo1t_d[c * 128:c * 128 + w, :])
            wo1s.append(to)
        wa2s, wo2s = [], []
        for c in range(2):
            ta = singles.tile([128, 6], FP, tag=f"wa2_{c}")
            nc.sync.dma_start(out=ta[:], in_=wa2t_d[c * 128:(c + 1) * 128, :])
            wa2s.append(ta)
            to = singles.tile([128, 18], FP, tag=f"wo2_{c}")
            nc.sync.dma_start(out=to[:], in_=wo2t_d[c * 128:(c + 1) * 128, :])
            wo2s.append(to)

        ident = singles.tile([128, 128], FP, tag="ident")
        make_identity(nc, ident[:])
        eps_t = singles.tile([128, 1], FP, tag="eps")
        nc.vector.memset(eps_t[:], float(EPS))

        def layernorm_relu(out_ap, x, g, be, n, tag):
            """out_ap[:, :n] = relu(LN(x) * g + be); x is (128, n) sbuf."""
            st6 = small.tile([128, 6], FP, tag=f"st6_{tag}")
            nc.vector.bn_stats(out=st6[:], in_=x[:])
            mv = small.tile([128, 2], FP, tag=f"mv_{tag}")
            nc.vector.bn_aggr(out=mv[:], in_=st6[:])
            std = small.tile([128, 1], FP, tag=f"std_{tag}")
            nc.scalar.activation(out=std[:], in_=mv[:, 1:2],
                                 func=mybir.ActivationFunctionType.Sqrt,
                                 bias=eps_t[:])
            rstd = small.tile([128, 1], FP, tag=f"rstd_{tag}")
            nc.vector.reciprocal(out=rstd[:], in_=std[:])
            xh = small.tile([128, n], FP, tag=f"xh_{tag}")
            nc.vector.tensor_scalar(out=xh[:], in0=x[:], scalar1=mv[:, 0:1],
                                    scalar2=rstd[:],
                                    op0=mybir.AluOpType.subtract,
                                    op1=mybir.AluOpType.mult)
            nc.vector.tensor_tensor(out=xh[:], in0=xh[:], in1=g[:],
                                    op=mybir.AluOpType.mult)
            nc.vector.tensor_tensor(out=xh[:], in0=xh[:], in1=be[:],
                                    op=mybir.AluOpType.add)
            nc.vector.tensor_relu(out=out_ap, in_=xh[:])

        for sb in range(SB):
            s0 = sb * 128
            # x = concat(relu(LN(features @ W1.T + b1)), info @ Wi.T + bi)
            x1 = small.tile([128, 512], FP, tag="x1")
            nc.sync.dma_start(out=x1[:], in_=ar_out[s0:s0 + 128, :])
            nc.vector.tensor_tensor(out=x1[:], in0=x1[:], in1=b1b[:],
                                    op=mybir.AluOpType.add)
            x_sb = singles.tile([128, 544], FP, tag=f"x_{sb}")
            layernorm_relu(x_sb[:, 0:512], x1, g1b, be1b, 512, f"pf{sb}")

            pinf = psum.tile([128, 32], FP, tag="pinf")
            nc.tensor.matmul(pinf[:], lhsT=infoT[:, s0:s0 + 128], rhs=wit[:],
                             start=True, stop=True)
            nc.vector.tensor_tensor(out=x_sb[:, 512:544], in0=pinf[:],
                                    in1=bib[:], op=mybir.AluOpType.add)

            # xT chunks for the 544-contraction matmuls
            xTs = []
            for c in range(5):
                w = 128 if c < 4 else 32
                pt = psum.tile([w, 128], FP, tag="pt")
                nc.tensor.transpose(out=pt[:], in_=x_sb[:, c * 128:c * 128 + w],
                                    identity=ident[:])
                xT_c = singles.tile([w, 128], FP, tag=f"xT_{sb}_{c}")
                nc.scalar.activation(out=xT_c[:], in_=pt[:],
                                     func=mybir.ActivationFunctionType.Copy)
                xTs.append(xT_c)

            def head(w1tiles, bb, g, be, w2tiles, b2b, ncols, out_dram, tag):
                ph = psum.tile([128, 256], FP, tag="ph")
                for c in range(5):
                    nc.tensor.matmul(ph[:], lhsT=xTs[c][:], rhs=w1tiles[c][:],
                                     start=(c == 0), stop=(c == 4))
                h = small.tile([128, 256], FP, tag=f"h_{tag}")
                nc.vector.tensor_tensor(out=h[:], in0=ph[:], in1=bb[:],
                                        op=mybir.AluOpType.add)
                hr = small.tile([128, 256], FP, tag=f"hr_{tag}")
                layernorm_relu(hr[:], h, g, be, 256, f"h{tag}{sb}")
                pp = psum.tile([128, 32], FP, tag="pp")
                for c in range(2):
                    pt2 = psum.tile([128, 128], FP, tag="pt")
                    nc.tensor.transpose(out=pt2[:], in_=hr[:, c * 128:(c + 1) * 128],
                                        identity=ident[:])
                    hT = small.tile([128, 128], FP, tag=f"hT_{tag}")
                    nc.scalar.activation(out=hT[:], in_=pt2[:],
                                         func=mybir.ActivationFunctionType.Copy)
                    nc.tensor.matmul(pp[:, 0:ncols], lhsT=hT[:], rhs=w2tiles[c][:],
                                     start=(c == 0), stop=(c == 1))
                outt = small.tile([128, ncols], FP, tag=f"o_{tag}")
                nc.vector.tensor_tensor(out=outt[:], in0=pp[:, 0:ncols],
                                        in1=b2b[:], op=mybir.AluOpType.add)
                nc.sync.dma_start(out=out_dram[s0:s0 + 128, :], in_=outt[:])

            head(wa1s, ba1b, gab, beab, wa2s, ba2b, 6, pred_d, "a")
            head(wo1s, bo1b, gob, beob, wo2s, bo2b, 18, off_d, "o")

    nc.compile()
    return nc


_PROG_CACHE = {}

# test harness hooks: set TRACE=True before calling kernel() to capture an
# NTFF profile; the measured NEFF time lands in LAST_EXEC_NS.
TRACE = False
LAST_EXEC_NS = None
LAST_RESULTS = None


def _ensure_ntff_hook():
    """The image's antenv package lacks axon_hooks; synthesize it so
    run_bass_kernel_spmd(trace=True) can reach the NTFF profiler."""
    import sys
    import types
    try:
        from antenv.axon_hooks import get_axon_ntff_profile_hook  # noqa: F401
        return
    except ImportError:
        pass
    import antenv
    from trn_agent_boot.trn_boot import _ntff_profile_via_ctypes
    hook = _ntff_profile_via_ctypes("/opt/axon/libaxon_pjrt.so")
    m = types.ModuleType("antenv.axon_hooks")
    m.get_axon_ntff_profile_hook = lambda: hook
    m.set_axon_ntff_profile_hook = lambda h: None
    sys.modules["antenv.axon_hooks"] = m
    antenv.axon_hooks = m


def _get_program(bounds_t, n_points, cw):
    key = (bounds_t, n_points, cw)
    if key not in _PROG_CACHE:
        _PROG_CACHE[key] = _build_program(list(bounds_t), n_points, cw)
    return _PROG_CACHE[key]


def _make_in_maps(feat, info, offsets, wd, B):
    """Build the 8 per-core input maps. wd: dict of weight arrays."""
    n, c = feat.shape
    featT = np.ascontiguousarray(feat.T)                      # (C, N)
    w1T = np.ascontiguousarray(wd["W1"].T)                    # (C, 512)
    lens = np.diff(offsets, prepend=0)
    floor = np.where(lens < KPAD, 0.0, NEG).astype(np.float32)[None, :]
    infoT = np.zeros((4, B), np.float32)
    infoT[:3] = info.T
    wit = np.zeros((4, 32), np.float32)
    wit[:3] = wd["Wi"].T

    common = dict(
        floor=floor,
        b1=wd["b1"][None, :], ln1g=wd["ln1_g"][None, :], ln1b=wd["ln1_b"][None, :],
        infoT=infoT, wit=wit, bi=wd["bi"][None, :],
        wa1t=np.ascontiguousarray(wd["Wa1"].T), ba1=wd["ba1"][None, :],
        lnag=wd["lna_g"][None, :], lnab=wd["lna_b"][None, :],
        wa2t=np.ascontiguousarray(wd["Wa2"].T), ba2=wd["ba2"][None, :],
        wo1t=np.ascontiguousarray(wd["Wo1"].T), bo1=wd["bo1"][None, :],
        lnog=wd["lno_g"][None, :], lnob=wd["lno_b"][None, :],
        wo2t=np.ascontiguousarray(wd["Wo2"].T), bo2=wd["bo2"][None, :],
    )
    common = {k: np.ascontiguousarray(v, dtype=np.float32)
              for k, v in common.items()}
    in_maps = []
    for d in range(NCORES):
        m = dict(common)
        m["featT"] = featT[d * P:(d + 1) * P]
        m["w1t"] = np.ascontiguousarray(w1T[d * P:(d + 1) * P])
        in_maps.append(m)
    return in_maps


def kernel(**inputs):
    xs = {k: np.asarray(v) for k, v in inputs.items()}
    feat = np.ascontiguousarray(xs["feat"], dtype=np.float32)
    info = np.ascontiguousarray(xs["info"], dtype=np.float32)
    offsets = np.asarray(xs["offsets"]).astype(np.int64)
    n, c = feat.shape
    B = offsets.shape[0]
    assert c == NCORES * P

    bounds = np.concatenate([[0], offsets]).astype(np.int64)
    cw = 4096
    nc = _get_program(tuple(int(v) for v in bounds), n, cw)
    in_maps = _make_in_maps(feat, info, offsets, xs, B)

    if TRACE:
        _ensure_ntff_hook()
        import concourse.bass_utils as _bu
        _bu.upload_artifacts = lambda d: d  # no S3 in this container
    res = run_bass_kernel_spmd(nc, in_maps, core_ids=list(range(NCORES)),
                               trace=TRACE)
    global LAST_EXEC_NS, LAST_RESULTS
    LAST_EXEC_NS = res.exec_time_ns
    LAST_RESULTS = res
    featuresT = np.concatenate(
        [res.results[d]["featT_out"] for d in range(NCORES)], axis=0)  # (C, B)
    features = np.ascontiguousarray(featuresT.T)
    pred = res.results[0]["pred_out"]
    offset = res.results[0]["off_out"].reshape(B, 6, 3)
    return features, pred, offset


# revision 15
# speedup vs baseline: 1.0889x; 1.0889x over previous
"""PointMultiGraspNet-V3 segment_reduce kernel for 8 Trainium2 NeuronCores.

Strategy: channel-parallel segment-max.  feat is transposed on host to
(C=1024, N=65536); core d owns channels [128d, 128d+128) for ALL points, so
the per-segment reduce ranges (derived from `offsets` at build time) are
identical on every core and the SPMD program is fully static.  Each core
streams its (128, N) slice from HBM and reduces each segment along the free
dim -- the kernel is HBM-bandwidth bound (32 MiB/core).

Per-segment reduction uses tensor_tensor_reduce with the range folded in
half (max(left, right) elementwise, then max-reduce, initial value = -3e38
or the running accumulator) -- 2 elements/cycle on DVE instead of 1 for a
plain reduce_max.

The MLP head needs all 1024 channels, so each core computes its partial
features @ W1.T (contraction over its own 128 channels) and the partials are
AllReduce-summed.  The segment dim is split into Q=4 quarters, each quarter's
partial matmul + AllReduce is issued as soon as the stream passes its last
point, so all but the last collective overlap the remaining streaming.  The
rest of the MLP is replicated on every core.  `features` itself is assembled
on host from the per-core channel slices; pred/offset are taken from core 0.
"""

import math
from contextlib import ExitStack

import numpy as np

import concourse.bacc as bacc
import concourse.bass as bass
import concourse.tile as tile
from concourse import mybir
from concourse.bass_utils import run_bass_kernel_spmd
from concourse.masks import make_identity

NCORES = 8
STAGED_COLLECTIVES = True
P = 128            # SBUF partitions == channels per core
EPS = 1e-5
KPAD = 512         # padded slots per segment in the original model
NEG = -3.0e38      # -inf stand-in (finite so 0*x etc. stay finite)
FP = mybir.dt.float32
AXX = mybir.AxisListType.X
MAX = mybir.AluOpType.max
ADD = mybir.AluOpType.add
SUB = mybir.AluOpType.subtract
MUL = mybir.AluOpType.mult
ACT_COPY = mybir.ActivationFunctionType.Copy


def _build_program(bounds, n_points, cw):
    """Build the SPMD Bass program.

    bounds: sequence of B+1 ints, bounds[0] == 0, bounds[-1] == n_points;
            segment s covers points [bounds[s], bounds[s+1]).
    """
    B = len(bounds) - 1
    assert B % 128 == 0
    SB = B // 128                  # 128-segment blocks (2 for B=256)
    Q = SB * 2                     # accumulator quarters (64 segments each)
    SQ = B // Q
    staged = STAGED_COLLECTIVES

    nc = bacc.Bacc("TRN2", target_bir_lowering=False, debug=False,
                   num_devices=NCORES)

    # ---- DRAM I/O -------------------------------------------------------
    featT = nc.dram_tensor("featT", [P, n_points], FP, kind="ExternalInput")
    floor_d = nc.dram_tensor("floor", [1, B], FP, kind="ExternalInput")
    w1t_d = nc.dram_tensor("w1t", [P, 512], FP, kind="ExternalInput")
    b1_d = nc.dram_tensor("b1", [1, 512], FP, kind="ExternalInput")
    ln1g_d = nc.dram_tensor("ln1g", [1, 512], FP, kind="ExternalInput")
    ln1b_d = nc.dram_tensor("ln1b", [1, 512], FP, kind="ExternalInput")
    infoT_d = nc.dram_tensor("infoT", [4, B], FP, kind="ExternalInput")
    wit_d = nc.dram_tensor("wit", [4, 32], FP, kind="ExternalInput")
    bi_d = nc.dram_tensor("bi", [1, 32], FP, kind="ExternalInput")
    wa1t_d = nc.dram_tensor("wa1t", [544, 256], FP, kind="ExternalInput")
    ba1_d = nc.dram_tensor("ba1", [1, 256], FP, kind="ExternalInput")
    lnag_d = nc.dram_tensor("lnag", [1, 256], FP, kind="ExternalInput")
    lnab_d = nc.dram_tensor("lnab", [1, 256], FP, kind="ExternalInput")
    wa2t_d = nc.dram_tensor("wa2t", [256, 6], FP, kind="ExternalInput")
    ba2_d = nc.dram_tensor("ba2", [1, 6], FP, kind="ExternalInput")
    wo1t_d = nc.dram_tensor("wo1t", [544, 256], FP, kind="ExternalInput")
    bo1_d = nc.dram_tensor("bo1", [1, 256], FP, kind="ExternalInput")
    lnog_d = nc.dram_tensor("lnog", [1, 256], FP, kind="ExternalInput")
    lnob_d = nc.dram_tensor("lnob", [1, 256], FP, kind="ExternalInput")
    wo2t_d = nc.dram_tensor("wo2t", [256, 18], FP, kind="ExternalInput")
    bo2_d = nc.dram_tensor("bo2", [1, 18], FP, kind="ExternalInput")

    featTout_d = nc.dram_tensor("featT_out", [P, B], FP, kind="ExternalOutput")
    pred_d = nc.dram_tensor("pred_out", [B, 6], FP, kind="ExternalOutput")
    off_d = nc.dram_tensor("off_out", [B, 18], FP, kind="ExternalOutput")

    nchunks = (n_points + cw - 1) // cw
    # chunk index after which quarter q's segments are complete
    q_done_chunk = [min((bounds[(q + 1) * SQ] - 1) // cw, nchunks - 1) if
                    bounds[(q + 1) * SQ] > 0 else 0 for q in range(Q)]

    with ExitStack() as ctx:
        tc = ctx.enter_context(tile.TileContext(nc))
        singles = ctx.enter_context(tc.tile_pool(name="singles", bufs=1))
        chunks = ctx.enter_context(tc.tile_pool(name="chunks", bufs=3))
        small = ctx.enter_context(tc.tile_pool(name="small", bufs=2))
        psum = ctx.enter_context(tc.tile_pool(name="psum", bufs=1, space="PSUM"))
        dram = ctx.enter_context(tc.tile_pool(name="dram", bufs=1, space="DRAM"))

        def bcast(name, src_ap, n):
            t = singles.tile([P, n], FP, tag=name)
            nc.sync.dma_start(out=t[:], in_=src_ap.to_broadcast([P, n]))
            return t

        # ---- persistent tiles ------------------------------------------
        acc_q = []
        for q in range(Q):
            a = singles.tile([P, SQ], FP, tag=f"acc{q}")
            nc.vector.memset(a[:], NEG)
            acc_q.append(a)
        fl_q = []
        for q in range(Q):
            fl_q.append(bcast(f"fl{q}", floor_d[:, q * SQ:(q + 1) * SQ], SQ))
        w1sb = singles.tile([P, 512], FP, tag="w1sb")
        nc.sync.dma_start(out=w1sb[:], in_=w1t_d[:, :])

        ar_in_all = dram.tile([B, 512], FP, name="ar_in_all", tag="ar_in_all")
        ar_out_all = dram.tile([B, 512], FP, name="ar_out_all",
                               tag="ar_out_all")
        ar_in = [ar_in_all[q * SQ:(q + 1) * SQ, :] for q in range(Q)]
        ar_out = [ar_out_all[q * SQ:(q + 1) * SQ, :] for q in range(Q)]

        def seg_reduce(t, la, lb, s, first):
            """max-reduce chunk-tile columns [la,lb) into segment s's slot."""
            q, col = s // SQ, s % SQ
            acc_col = acc_q[q][:, col:col + 1]
            if first:
                nc.vector.reduce_max(out=acc_col, in_=t[:, la:lb], axis=AXX)
            else:
                tmp = small.tile([P, 1], FP, tag="tmp")
                nc.vector.reduce_max(out=tmp[:], in_=t[:, la:lb], axis=AXX)
                nc.vector.tensor_max(out=acc_col, in0=acc_col, in1=tmp[:])

        def stage_tail(q):
            """fT_q = clamp(acc_q); partial W1 matmul; AllReduce quarter q."""
            fTq = singles.tile([P, SQ], FP, tag=f"fT{q}")
            nc.vector.tensor_max(out=fTq[:], in0=acc_q[q][:], in1=fl_q[q][:])
            nc.sync.dma_start(out=featTout_d[:, q * SQ:(q + 1) * SQ], in_=fTq[:])
            p1 = psum.tile([SQ, 512], FP, tag="p1")
            nc.tensor.matmul(p1[:], lhsT=fTq[:], rhs=w1sb[:],
                             start=True, stop=True)
            cp = small.tile([SQ, 512], FP, tag="cp")
            eng = nc.scalar if q < Q - 1 else nc.vector
            if q < Q - 1:
                nc.scalar.activation(out=cp[:], in_=p1[:], func=ACT_COPY)
            else:
                nc.vector.tensor_copy(out=cp[:], in_=p1[:])
            nc.sync.dma_start(out=ar_in[q], in_=cp[:])
            if staged:
                nc.gpsimd.collective_compute(
                    "AllReduce", ADD, replica_groups=[list(range(NCORES))],
                    ins=[ar_in[q].opt()], outs=[ar_out[q].opt()])

        # ---- stream + segment reduce -----------------------------------
        stage_q = 0
        for ci in range(nchunks):
            c0 = ci * cw
            c1 = min(n_points, c0 + cw)
            w = c1 - c0
            t = chunks.tile([P, cw], FP, tag="chunk")
            eng = nc.sync if ci % 2 == 0 else nc.scalar
            eng.dma_start(out=t[:, :w], in_=featT[:, c0:c1])
            for s in range(B):
                a = max(bounds[s], c0)
                b = min(bounds[s + 1], c1)
                if b <= a:
                    continue
                seg_reduce(t, a - c0, b - c0, s, bounds[s] >= c0)
            while stage_q < Q and q_done_chunk[stage_q] == ci:
                stage_tail(stage_q)
                stage_q += 1
        while stage_q < Q:      # quarters with no points still need output
            stage_tail(stage_q)
            stage_q += 1
        if not staged:
            nc.gpsimd.collective_compute(
                "AllReduce", ADD, replica_groups=[list(range(NCORES))],
                ins=[ar_in_all[:].opt()], outs=[ar_out_all[:].opt()])

        # ---- MLP head ---------------------------------------------------
        b1b = bcast("b1b", b1_d[:], 512)
        g1b = bcast("g1b", ln1g_d[:], 512)
        be1b = bcast("be1b", ln1b_d[:], 512)
        bib = bcast("bib", bi_d[:], 32)
        ba1b = bcast("ba1b", ba1_d[:], 256)
        gab = bcast("gab", lnag_d[:], 256)
        beab = bcast("beab", lnab_d[:], 256)
        ba2b = bcast("ba2b", ba2_d[:], 6)
        bo1b = bcast("bo1b", bo1_d[:], 256)
        gob = bcast("gob", lnog_d[:], 256)
        beob = bcast("beob", lnob_d[:], 256)
        bo2b = bcast("bo2b", bo2_d[:], 18)

        infoT = singles.tile([4, B], FP, tag="infoT")
        nc.sync.dma_start(out=infoT[:], in_=infoT_d[:, :])
        wit = singles.tile([4, 32], FP, tag="wit")
        nc.sync.dma_start(out=wit[:], in_=wit_d[:, :])
        wa1s, wo1s = [], []
        for c in range(5):
            w = 128 if c < 4 else 32
            ta = singles.tile([w, 256], FP, tag=f"wa1_{c}")
            nc.sync.dma_start(out=ta[:], in_=wa1t_d[c * 128:c * 128 + w, :])
            wa1s.append(ta)
            to = singles.tile([w, 256], FP, tag=f"wo1_{c}")
            nc.sync.dma_start(out=to[:], in_=wo1t_d[c * 128:c * 128 + w, :])
            wo1s.append(to)
        wa2s, wo2s = [], []
        for c in range(2):
            ta = singles.tile([128, 6], FP, tag=f"wa2_{c}")
            nc.sync.dma_start(out=ta[:], in_=wa2t_d[c * 128:(c + 1) * 128, :])
            wa2s.append(ta)
            to = singles.tile([128, 18], FP, tag=f"wo2_{c}")
            nc.sync.dma_start(out=to[:], in_=wo2t_d[c * 128:(c + 1) * 128, :])
            wo2s.append(to)

        ident = singles.tile([128, 128], FP, tag="ident")
        make_identity(nc, ident[:])
        eps_t = singles.tile([128, 1], FP, tag="eps")
        nc.vector.memset(eps_t[:], float(EPS))

        def layernorm_relu(out_ap, x, g, be, n, tag):
            """out_ap = relu(LN(x) * g + be); x is (128, n) sbuf."""
            st6 = small.tile([128, 6], FP, tag=f"st6_{tag}")
            nc.vector.bn_stats(out=st6[:], in_=x[:])
            mv = small.tile([128, 2], FP, tag=f"mv_{tag}")
            nc.vector.bn_aggr(out=mv[:], in_=st6[:])
            std = small.tile([128, 1], FP, tag=f"std_{tag}")
            nc.scalar.activation(out=std[:], in_=mv[:, 1:2],
                                 func=mybir.ActivationFunctionType.Sqrt,
                                 bias=eps_t[:])
            rstd = small.tile([128, 1], FP, tag=f"rstd_{tag}")
            nc.vector.reciprocal(out=rstd[:], in_=std[:])
            xh = small.tile([128, n], FP, tag=f"xh_{tag}")
            nc.vector.tensor_scalar(out=xh[:], in0=x[:], scalar1=mv[:, 0:1],
                                    scalar2=rstd[:], op0=SUB, op1=MUL)
            nc.vector.tensor_tensor(out=xh[:], in0=xh[:], in1=g[:], op=MUL)
            nc.vector.tensor_tensor(out=xh[:], in0=xh[:], in1=be[:], op=ADD)
            nc.vector.tensor_relu(out=out_ap, in_=xh[:])

        for sb in range(SB):
            s0 = sb * 128
            # x = concat(relu(LN(features @ W1.T + b1)), info @ Wi.T + bi)
            x1 = small.tile([128, 512], FP, tag="x1")
            nc.sync.dma_start(out=x1[0:SQ, :], in_=ar_out[2 * sb])
            nc.sync.dma_start(out=x1[SQ:128, :], in_=ar_out[2 * sb + 1])
            nc.vector.tensor_tensor(out=x1[:], in0=x1[:], in1=b1b[:], op=ADD)
            x_sb = singles.tile([128, 544], FP, tag=f"x_{sb}")
            layernorm_relu(x_sb[:, 0:512], x1, g1b, be1b, 512, f"pf{sb}")

            pinf = psum.tile([128, 32], FP, tag="pinf")
            nc.tensor.matmul(pinf[:], lhsT=infoT[:, s0:s0 + 128], rhs=wit[:],
                             start=True, stop=True)
            nc.vector.tensor_tensor(out=x_sb[:, 512:544], in0=pinf[:],
                                    in1=bib[:], op=ADD)

            # xT chunks for the 544-contraction matmuls
            xTs = []
            for c in range(5):
                w = 128 if c < 4 else 32
                pt = psum.tile([w, 128], FP, tag="pt")
                nc.tensor.transpose(out=pt[:], in_=x_sb[:, c * 128:c * 128 + w],
                                    identity=ident[:])
                xT_c = singles.tile([w, 128], FP, tag=f"xT_{sb}_{c}")
                nc.vector.tensor_copy(out=xT_c[:], in_=pt[:])
                xTs.append(xT_c)

            def head(w1tiles, bb, g, be, w2tiles, b2b, ncols, out_dram, tag):
                ph = psum.tile([128, 256], FP, tag="ph")
                for c in range(5):
                    nc.tensor.matmul(ph[:], lhsT=xTs[c][:], rhs=w1tiles[c][:],
                                     start=(c == 0), stop=(c == 4))
                h = small.tile([128, 256], FP, tag=f"h_{tag}")
                nc.vector.tensor_tensor(out=h[:], in0=ph[:], in1=bb[:], op=ADD)
                hr = small.tile([128, 256], FP, tag=f"hr_{tag}")
                layernorm_relu(hr[:], h, g, be, 256, f"h{tag}{sb}")
                pp = psum.tile([128, 32], FP, tag="pp")
                for c in range(2):
                    pt2 = psum.tile([128, 128], FP, tag="pt")
                    nc.tensor.transpose(out=pt2[:],
                                        in_=hr[:, c * 128:(c + 1) * 128],
                                        identity=ident[:])
                    hT = small.tile([128, 128], FP, tag=f"hT_{tag}")
                    nc.vector.tensor_copy(out=hT[:], in_=pt2[:])
                    nc.tensor.matmul(pp[:, 0:ncols], lhsT=hT[:],
                                     rhs=w2tiles[c][:],
                                     start=(c == 0), stop=(c == 1))
                outt = small.tile([128, ncols], FP, tag=f"o_{tag}")
                nc.vector.tensor_tensor(out=outt[:], in0=pp[:, 0:ncols],
                                        in1=b2b[:], op=ADD)
                nc.sync.dma_start(out=out_dram[s0:s0 + 128, :], in_=outt[:])

            head(wa1s, ba1b, gab, beab, wa2s, ba2b, 6, pred_d, "a")
            head(wo1s, bo1b, gob, beob, wo2s, bo2b, 18, off_d, "o")

    nc.compile()
    return nc


_PROG_CACHE = {}

# test harness hooks: set TRACE=True before calling kernel() to capture an
# NTFF profile; the measured NEFF time lands in LAST_EXEC_NS.
TRACE = False
LAST_EXEC_NS = None
LAST_RESULTS = None


def _ensure_ntff_hook():
    """The image's antenv package lacks axon_hooks; synthesize it so
    run_bass_kernel_spmd(trace=True) can reach the NTFF profiler."""
    import sys
    import types
    try:
        from antenv.axon_hooks import get_axon_ntff_profile_hook  # noqa: F401
        return
    except ImportError:
        pass
    import antenv
    from trn_agent_boot.trn_boot import _ntff_profile_via_ctypes
    hook = _ntff_profile_via_ctypes("/opt/axon/libaxon_pjrt.so")
    m = types.ModuleType("antenv.axon_hooks")
    m.get_axon_ntff_profile_hook = lambda: hook
    m.set_axon_ntff_profile_hook = lambda h: None
    sys.modules["antenv.axon_hooks"] = m
    antenv.axon_hooks = m


def _get_program(bounds_t, n_points, cw):
    key = (bounds_t, n_points, cw)
    if key not in _PROG_CACHE:
        _PROG_CACHE[key] = _build_program(list(bounds_t), n_points, cw)
    return _PROG_CACHE[key]


def _make_in_maps(feat, info, offsets, wd, B):
    """Build the 8 per-core input maps. wd: dict of weight arrays."""
    featT = np.ascontiguousarray(feat.T)                      # (C, N)
    w1T = np.ascontiguousarray(wd["W1"].T)                    # (C, 512)
    lens = np.diff(offsets, prepend=0)
    floor = np.where(lens < KPAD, 0.0, NEG).astype(np.float32)[None, :]
    infoT = np.zeros((4, B), np.float32)
    infoT[:3] = info.T
    wit = np.zeros((4, 32), np.float32)
    wit[:3] = wd["Wi"].T

    common = dict(
        floor=floor,
        b1=wd["b1"][None, :], ln1g=wd["ln1_g"][None, :], ln1b=wd["ln1_b"][None, :],
        infoT=infoT, wit=wit, bi=wd["bi"][None, :],
        wa1t=np.ascontiguousarray(wd["Wa1"].T), ba1=wd["ba1"][None, :],
        lnag=wd["lna_g"][None, :], lnab=wd["lna_b"][None, :],
        wa2t=np.ascontiguousarray(wd["Wa2"].T), ba2=wd["ba2"][None, :],
        wo1t=np.ascontiguousarray(wd["Wo1"].T), bo1=wd["bo1"][None, :],
        lnog=wd["lno_g"][None, :], lnob=wd["lno_b"][None, :],
        wo2t=np.ascontiguousarray(wd["Wo2"].T), bo2=wd["bo2"][None, :],
    )
    common = {k: np.ascontiguousarray(v, dtype=np.float32)
              for k, v in common.items()}
    in_maps = []
    for d in range(NCORES):
        m = dict(common)
        m["featT"] = featT[d * P:(d + 1) * P]
        m["w1t"] = np.ascontiguousarray(w1T[d * P:(d + 1) * P])
        in_maps.append(m)
    return in_maps


def kernel(**inputs):
    xs = {k: np.asarray(v) for k, v in inputs.items()}
    feat = np.ascontiguousarray(xs["feat"], dtype=np.float32)
    info = np.ascontiguousarray(xs["info"], dtype=np.float32)
    offsets = np.asarray(xs["offsets"]).astype(np.int64)
    n, c = feat.shape
    B = offsets.shape[0]
    assert c == NCORES * P

    bounds = np.concatenate([[0], offsets]).astype(np.int64)
    cw = 4096
    nc = _get_program(tuple(int(v) for v in bounds), n, cw)
    in_maps = _make_in_maps(feat, info, offsets, xs, B)

    if TRACE:
        _ensure_ntff_hook()
        import concourse.bass_utils as _bu
        _bu.upload_artifacts = lambda d: d  # no S3 in this container
    res = run_bass_kernel_spmd(nc, in_maps, core_ids=list(range(NCORES)),
                               trace=TRACE)
    global LAST_EXEC_NS, LAST_RESULTS
    LAST_EXEC_NS = res.exec_time_ns
    LAST_RESULTS = res
    featuresT = np.concatenate(
        [res.results[d]["featT_out"] for d in range(NCORES)], axis=0)  # (C, B)
    features = np.ascontiguousarray(featuresT.T)
    pred = res.results[0]["pred_out"]
    offset = res.results[0]["off_out"].reshape(B, 6, 3)
    return features, pred, offset


# revision 17
# speedup vs baseline: 1.2135x; 1.1144x over previous
"""PointMultiGraspNet-V3 segment_reduce kernel for 8 Trainium2 NeuronCores.

Strategy: channel-parallel segment-max.  feat is transposed on host to
(C=1024, N=65536); core d owns channels [128d, 128d+128) for ALL points, so
the per-segment reduce ranges (derived from `offsets` at build time) are
identical on every core and the SPMD program is fully static.  Each core
streams its (128, N) slice from HBM and reduces each segment along the free
dim -- the kernel is HBM-bandwidth bound (32 MiB/core).

The MLP head needs all 1024 channels, so each core computes its partial
features @ W1.T (contraction over its own 128 channels) and the partials are
AllReduce-summed.  The segment dim is split into Q=4 quarters; each quarter's
partial matmul + AllReduce is issued as soon as the stream passes its last
point, so all but the last collective overlap the remaining streaming, and
the first segment-block's MLP chain is emitted mid-stream so it overlaps the
tail of the stream too.  The rest of the MLP is replicated on every core.
`features` is assembled on host from the per-core channel slices;
pred/offset are taken from core 0.
"""

from contextlib import ExitStack

import numpy as np

import concourse.bacc as bacc
import concourse.tile as tile
from concourse import mybir
from concourse.bass_utils import run_bass_kernel_spmd
from concourse.masks import make_identity

NCORES = 8
P = 128            # SBUF partitions == channels per core
EPS = 1e-5
KPAD = 512         # padded slots per segment in the original model
NEG = -3.0e38      # -inf stand-in (finite so 0*x etc. stay finite)
FP = mybir.dt.float32
AXX = mybir.AxisListType.X
MAX = mybir.AluOpType.max
ADD = mybir.AluOpType.add
SUB = mybir.AluOpType.subtract
MUL = mybir.AluOpType.mult
ACT_COPY = mybir.ActivationFunctionType.Copy


def _build_program(bounds, n_points, cw, gb_id):
    """Build the SPMD Bass program.

    bounds: sequence of B+1 ints, bounds[0] == 0, bounds[-1] == n_points;
            segment s covers points [bounds[s], bounds[s+1]).
    gb_id: (ln1, lna, lno) -- True when that LayerNorm's gamma/beta are
           exactly ones/zeros, allowing the scale/shift ops to be skipped.
    """
    B = len(bounds) - 1
    assert B % 128 == 0
    SB = B // 128                  # 128-segment blocks (2 for B=256)
    Q = SB * 2                     # accumulator quarters (64 segments each)
    SQ = B // Q

    nc = bacc.Bacc("TRN2", target_bir_lowering=False, debug=False,
                   num_devices=NCORES)

    # ---- DRAM I/O -------------------------------------------------------
    featT = nc.dram_tensor("featT", [P, n_points], FP, kind="ExternalInput")
    floor_d = nc.dram_tensor("floor", [1, B], FP, kind="ExternalInput")
    w1t_d = nc.dram_tensor("w1t", [P, 512], FP, kind="ExternalInput")
    b1_d = nc.dram_tensor("b1", [1, 512], FP, kind="ExternalInput")
    ln1g_d = nc.dram_tensor("ln1g", [1, 512], FP, kind="ExternalInput")
    ln1b_d = nc.dram_tensor("ln1b", [1, 512], FP, kind="ExternalInput")
    infoT_d = nc.dram_tensor("infoT", [4, B], FP, kind="ExternalInput")
    wit_d = nc.dram_tensor("wit", [4, 32], FP, kind="ExternalInput")
    bi_d = nc.dram_tensor("bi", [1, 32], FP, kind="ExternalInput")
    wa1t_d = nc.dram_tensor("wa1t", [544, 256], FP, kind="ExternalInput")
    ba1_d = nc.dram_tensor("ba1", [1, 256], FP, kind="ExternalInput")
    lnag_d = nc.dram_tensor("lnag", [1, 256], FP, kind="ExternalInput")
    lnab_d = nc.dram_tensor("lnab", [1, 256], FP, kind="ExternalInput")
    wa2t_d = nc.dram_tensor("wa2t", [256, 6], FP, kind="ExternalInput")
    ba2_d = nc.dram_tensor("ba2", [1, 6], FP, kind="ExternalInput")
    wo1t_d = nc.dram_tensor("wo1t", [544, 256], FP, kind="ExternalInput")
    bo1_d = nc.dram_tensor("bo1", [1, 256], FP, kind="ExternalInput")
    lnog_d = nc.dram_tensor("lnog", [1, 256], FP, kind="ExternalInput")
    lnob_d = nc.dram_tensor("lnob", [1, 256], FP, kind="ExternalInput")
    wo2t_d = nc.dram_tensor("wo2t", [256, 18], FP, kind="ExternalInput")
    bo2_d = nc.dram_tensor("bo2", [1, 18], FP, kind="ExternalInput")

    featTout_d = nc.dram_tensor("featT_out", [P, B], FP, kind="ExternalOutput")
    pred_d = nc.dram_tensor("pred_out", [B, 6], FP, kind="ExternalOutput")
    off_d = nc.dram_tensor("off_out", [B, 18], FP, kind="ExternalOutput")

    nchunks = (n_points + cw - 1) // cw
    # chunk index after which quarter q's segments are complete
    q_done_chunk = [min((bounds[(q + 1) * SQ] - 1) // cw, nchunks - 1) if
                    bounds[(q + 1) * SQ] > 0 else 0 for q in range(Q)]

    with ExitStack() as ctx:
        tc = ctx.enter_context(tile.TileContext(nc))
        singles = ctx.enter_context(tc.tile_pool(name="singles", bufs=1))
        chunks = ctx.enter_context(tc.tile_pool(name="chunks", bufs=4))
        small = ctx.enter_context(tc.tile_pool(name="small", bufs=2))
        psum = ctx.enter_context(tc.tile_pool(name="psum", bufs=1, space="PSUM"))
        dram = ctx.enter_context(tc.tile_pool(name="dram", bufs=1, space="DRAM"))

        def bcast(name, src_ap, n):
            t = singles.tile([P, n], FP, tag=name)
            nc.scalar.dma_start(out=t[:], in_=src_ap.to_broadcast([P, n]))
            return t

        # ---- persistent tiles / weight loads (up front) -----------------
        acc_q = []
        for q in range(Q):
            a = singles.tile([P, SQ], FP, tag=f"acc{q}")
            nc.vector.memset(a[:], NEG)
            acc_q.append(a)
        fl_q = []
        for q in range(Q):
            fl_q.append(bcast(f"fl{q}", floor_d[:, q * SQ:(q + 1) * SQ], SQ))
        w1sb = singles.tile([P, 512], FP, tag="w1sb")
        nc.scalar.dma_start(out=w1sb[:], in_=w1t_d[:, :])

        b1b = bcast("b1b", b1_d[:], 512)
        g1b = None if gb_id[0] else bcast("g1b", ln1g_d[:], 512)
        be1b = None if gb_id[0] else bcast("be1b", ln1b_d[:], 512)
        bib = bcast("bib", bi_d[:], 32)
        ba1b = bcast("ba1b", ba1_d[:], 256)
        gab = None if gb_id[1] else bcast("gab", lnag_d[:], 256)
        beab = None if gb_id[1] else bcast("beab", lnab_d[:], 256)
        ba2b = bcast("ba2b", ba2_d[:], 6)
        bo1b = bcast("bo1b", bo1_d[:], 256)
        gob = None if gb_id[2] else bcast("gob", lnog_d[:], 256)
        beob = None if gb_id[2] else bcast("beob", lnob_d[:], 256)
        bo2b = bcast("bo2b", bo2_d[:], 18)

        infoT = singles.tile([4, B], FP, tag="infoT")
        nc.scalar.dma_start(out=infoT[:], in_=infoT_d[:, :])
        wit = singles.tile([4, 32], FP, tag="wit")
        nc.scalar.dma_start(out=wit[:], in_=wit_d[:, :])
        wa1s, wo1s = [], []
        for c in range(5):
            w = 128 if c < 4 else 32
            ta = singles.tile([w, 256], FP, tag=f"wa1_{c}")
            nc.scalar.dma_start(out=ta[:], in_=wa1t_d[c * 128:c * 128 + w, :])
            wa1s.append(ta)
            to = singles.tile([w, 256], FP, tag=f"wo1_{c}")
            nc.scalar.dma_start(out=to[:], in_=wo1t_d[c * 128:c * 128 + w, :])
            wo1s.append(to)
        wa2s, wo2s = [], []
        for c in range(2):
            ta = singles.tile([128, 6], FP, tag=f"wa2_{c}")
            nc.scalar.dma_start(out=ta[:], in_=wa2t_d[c * 128:(c + 1) * 128, :])
            wa2s.append(ta)
            to = singles.tile([128, 18], FP, tag=f"wo2_{c}")
            nc.scalar.dma_start(out=to[:], in_=wo2t_d[c * 128:(c + 1) * 128, :])
            wo2s.append(to)

        ident = singles.tile([128, 128], FP, tag="ident")
        make_identity(nc, ident[:])
        eps_t = singles.tile([128, 1], FP, tag="eps")
        nc.vector.memset(eps_t[:], float(EPS))

        ar_in_all = dram.tile([B, 512], FP, name="ar_in_all", tag="ar_in_all")
        ar_out_all = dram.tile([B, 512], FP, name="ar_out_all",
                               tag="ar_out_all")
        ar_in = [ar_in_all[q * SQ:(q + 1) * SQ, :] for q in range(Q)]
        ar_out = [ar_out_all[q * SQ:(q + 1) * SQ, :] for q in range(Q)]

        def seg_reduce(t, la, lb, s, first):
            """max-reduce chunk-tile columns [la,lb) into segment s's slot."""
            q, col = s // SQ, s % SQ
            acc_col = acc_q[q][:, col:col + 1]
            if first:
                nc.vector.reduce_max(out=acc_col, in_=t[:, la:lb], axis=AXX)
            else:
                tmp = small.tile([P, 1], FP, tag="tmp")
                nc.vector.reduce_max(out=tmp[:], in_=t[:, la:lb], axis=AXX)
                nc.vector.tensor_max(out=acc_col, in0=acc_col, in1=tmp[:])

        def stage_tail(q):
            """fT_q = clamp(acc_q); partial W1 matmul; AllReduce quarter q."""
            fTq = singles.tile([P, SQ], FP, tag=f"fT{q}")
            nc.vector.tensor_max(out=fTq[:], in0=acc_q[q][:], in1=fl_q[q][:])
            nc.sync.dma_start(out=featTout_d[:, q * SQ:(q + 1) * SQ], in_=fTq[:])
            p1 = psum.tile([SQ, 512], FP, tag="p1")
            nc.tensor.matmul(p1[:], lhsT=fTq[:], rhs=w1sb[:],
                             start=True, stop=True)
            cp = small.tile([SQ, 512], FP, tag="cp")
            if q < Q - 1:
                nc.scalar.activation(out=cp[:], in_=p1[:], func=ACT_COPY)
            else:
                nc.vector.tensor_copy(out=cp[:], in_=p1[:])
            nc.sync.dma_start(out=ar_in[q], in_=cp[:])
            nc.gpsimd.collective_compute(
                "AllReduce", ADD, replica_groups=[list(range(NCORES))],
                ins=[ar_in[q].opt()], outs=[ar_out[q].opt()])

        def layernorm_relu(out_ap, x, g, be, n, tag):
            """out_ap = relu(LN(x) * g + be); x is (128, n) sbuf."""
            st6 = small.tile([128, 6], FP, tag=f"st6_{tag}")
            nc.vector.bn_stats(out=st6[:], in_=x[:])
            mv = small.tile([128, 2], FP, tag=f"mv_{tag}")
            nc.vector.bn_aggr(out=mv[:], in_=st6[:])
            std = small.tile([128, 1], FP, tag=f"std_{tag}")
            nc.scalar.activation(out=std[:], in_=mv[:, 1:2],
                                 func=mybir.ActivationFunctionType.Sqrt,
                                 bias=eps_t[:])
            rstd = small.tile([128, 1], FP, tag=f"rstd_{tag}")
            nc.vector.reciprocal(out=rstd[:], in_=std[:])
            if g is None:
                # gamma==1, beta==0: (x - mean) * rstd, then relu, fused
                xh = small.tile([128, n], FP, tag=f"xh_{tag}")
                nc.vector.tensor_scalar(out=xh[:], in0=x[:],
                                        scalar1=mv[:, 0:1], scalar2=rstd[:],
                                        op0=SUB, op1=MUL)
                nc.vector.tensor_scalar_max(out=out_ap, in0=xh[:],
                                            scalar1=0.0)
            else:
                xh = small.tile([128, n], FP, tag=f"xh_{tag}")
                nc.vector.tensor_scalar(out=xh[:], in0=x[:],
                                        scalar1=mv[:, 0:1], scalar2=rstd[:],
                                        op0=SUB, op1=MUL)
                nc.vector.tensor_tensor(out=xh[:], in0=xh[:], in1=g[:], op=MUL)
                nc.vector.tensor_tensor(out=xh[:], in0=xh[:], in1=be[:], op=ADD)
                nc.vector.tensor_relu(out=out_ap, in_=xh[:])

        def mlp_block(sb):
            s0 = sb * 128
            # x = concat(relu(LN(features @ W1.T + b1)), info @ Wi.T + bi)
            x1 = small.tile([128, 512], FP, tag="x1")
            nc.sync.dma_start(out=x1[0:SQ, :], in_=ar_out[2 * sb])
            nc.sync.dma_start(out=x1[SQ:128, :], in_=ar_out[2 * sb + 1])
            nc.vector.tensor_tensor(out=x1[:], in0=x1[:], in1=b1b[:], op=ADD)
            x_sb = singles.tile([128, 544], FP, tag=f"x_{sb}")
            layernorm_relu(x_sb[:, 0:512], x1, g1b, be1b, 512, f"pf{sb}")

            pinf = psum.tile([128, 32], FP, tag="pinf")
            nc.tensor.matmul(pinf[:], lhsT=infoT[:, s0:s0 + 128], rhs=wit[:],
                             start=True, stop=True)
            nc.vector.tensor_tensor(out=x_sb[:, 512:544], in0=pinf[:],
                                    in1=bib[:], op=ADD)

            # xT chunks for the 544-contraction matmuls
            xTs = []
            for c in range(5):
                w = 128 if c < 4 else 32
                pt = psum.tile([w, 128], FP, tag="pt")
                nc.tensor.transpose(out=pt[:], in_=x_sb[:, c * 128:c * 128 + w],
                                    identity=ident[:])
                xT_c = singles.tile([w, 128], FP, tag=f"xT_{sb}_{c}")
                nc.vector.tensor_copy(out=xT_c[:], in_=pt[:])
                xTs.append(xT_c)

            def head(w1tiles, bb, g, be, w2tiles, b2b, ncols, out_dram, tag):
                ph = psum.tile([128, 256], FP, tag="ph")
                for c in range(5):
                    nc.tensor.matmul(ph[:], lhsT=xTs[c][:], rhs=w1tiles[c][:],
                                     start=(c == 0), stop=(c == 4))
                h = small.tile([128, 256], FP, tag=f"h_{tag}")
                nc.vector.tensor_tensor(out=h[:], in0=ph[:], in1=bb[:], op=ADD)
                hr = small.tile([128, 256], FP, tag=f"hr_{tag}")
                layernorm_relu(hr[:], h, g, be, 256, f"h{tag}{sb}")
                pp = psum.tile([128, 32], FP, tag="pp")
                for c in range(2):
                    pt2 = psum.tile([128, 128], FP, tag="pt")
                    nc.tensor.transpose(out=pt2[:],
                                        in_=hr[:, c * 128:(c + 1) * 128],
                                        identity=ident[:])
                    hT = small.tile([128, 128], FP, tag=f"hT_{tag}")
                    nc.vector.tensor_copy(out=hT[:], in_=pt2[:])
                    nc.tensor.matmul(pp[:, 0:ncols], lhsT=hT[:],
                                     rhs=w2tiles[c][:],
                                     start=(c == 0), stop=(c == 1))
                outt = small.tile([128, ncols], FP, tag=f"o_{tag}")
                nc.vector.tensor_tensor(out=outt[:], in0=pp[:, 0:ncols],
                                        in1=b2b[:], op=ADD)
                nc.sync.dma_start(out=out_dram[s0:s0 + 128, :], in_=outt[:])

            head(wa1s, ba1b, gab, beab, wa2s, ba2b, 6, pred_d, "a")
            head(wo1s, bo1b, gob, beob, wo2s, bo2b, 18, off_d, "o")

        # ---- stream + segment reduce + interleaved stage/MLP emission ---
        stage_q = 0
        mlp_done = 0
        for ci in range(nchunks):
            c0 = ci * cw
            c1 = min(n_points, c0 + cw)
            w = c1 - c0
            t = chunks.tile([P, cw], FP, tag="chunk")
            eng = nc.sync if ci % 2 == 0 else nc.scalar
            eng.dma_start(out=t[:, :w], in_=featT[:, c0:c1])
            for s in range(B):
                a = max(bounds[s], c0)
                b = min(bounds[s + 1], c1)
                if b <= a:
                    continue
                seg_reduce(t, a - c0, b - c0, s, bounds[s] >= c0)
            while stage_q < Q and q_done_chunk[stage_q] == ci:
                stage_tail(stage_q)
                stage_q += 1
            # emit seg-block sb's MLP once stage 2*sb+2 is also done: its
            # collectives are then long finished, so the chain never stalls
            # the in-order engine queues mid-stream.
            while mlp_done < SB and stage_q >= 2 * mlp_done + 3:
                mlp_block(mlp_done)
                mlp_done += 1
        while stage_q < Q:      # quarters with no points still need output
            stage_tail(stage_q)
            stage_q += 1
        while mlp_done < SB:
            mlp_block(mlp_done)
            mlp_done += 1

    nc.compile()
    return nc


_PROG_CACHE = {}

# test harness hooks: set TRACE=True before calling kernel() to capture an
# NTFF profile; the measured NEFF time lands in LAST_EXEC_NS.
TRACE = False
LAST_EXEC_NS = None
LAST_RESULTS = None


def _ensure_ntff_hook():
    """The image's antenv package lacks axon_hooks; synthesize it so
    run_bass_kernel_spmd(trace=True) can reach the NTFF profiler."""
    import sys
    import types
    try:
        from antenv.axon_hooks import get_axon_ntff_profile_hook  # noqa: F401
        return
    except ImportError:
        pass
    import antenv
    from trn_agent_boot.trn_boot import _ntff_profile_via_ctypes
    hook = _ntff_profile_via_ctypes("/opt/axon/libaxon_pjrt.so")
    m = types.ModuleType("antenv.axon_hooks")
    m.get_axon_ntff_profile_hook = lambda: hook
    m.set_axon_ntff_profile_hook = lambda h: None
    sys.modules["antenv.axon_hooks"] = m
    antenv.axon_hooks = m


def _get_program(bounds_t, n_points, cw, gb_id):
    key = (bounds_t, n_points, cw, gb_id)
    if key not in _PROG_CACHE:
        _PROG_CACHE[key] = _build_program(list(bounds_t), n_points, cw, gb_id)
    return _PROG_CACHE[key]


def _make_in_maps(feat, info, offsets, wd, B):
    """Build the 8 per-core input maps. wd: dict of weight arrays."""
    featT = np.ascontiguousarray(feat.T)                      # (C, N)
    w1T = np.ascontiguousarray(wd["W1"].T)                    # (C, 512)
    lens = np.diff(offsets, prepend=0)
    floor = np.where(lens < KPAD, 0.0, NEG).astype(np.float32)[None, :]
    infoT = np.zeros((4, B), np.float32)
    infoT[:3] = info.T
    wit = np.zeros((4, 32), np.float32)
    wit[:3] = wd["Wi"].T

    common = dict(
        floor=floor,
        b1=wd["b1"][None, :], ln1g=wd["ln1_g"][None, :], ln1b=wd["ln1_b"][None, :],
        infoT=infoT, wit=wit, bi=wd["bi"][None, :],
        wa1t=np.ascontiguousarray(wd["Wa1"].T), ba1=wd["ba1"][None, :],
        lnag=wd["lna_g"][None, :], lnab=wd["lna_b"][None, :],
        wa2t=np.ascontiguousarray(wd["Wa2"].T), ba2=wd["ba2"][None, :],
        wo1t=np.ascontiguousarray(wd["Wo1"].T), bo1=wd["bo1"][None, :],
        lnog=wd["lno_g"][None, :], lnob=wd["lno_b"][None, :],
        wo2t=np.ascontiguousarray(wd["Wo2"].T), bo2=wd["bo2"][None, :],
    )
    common = {k: np.ascontiguousarray(v, dtype=np.float32)
              for k, v in common.items()}
    in_maps = []
    for d in range(NCORES):
        m = dict(common)
        m["featT"] = featT[d * P:(d + 1) * P]
        m["w1t"] = np.ascontiguousarray(w1T[d * P:(d + 1) * P])
        in_maps.append(m)
    return in_maps


def _gb_identity(wd):
    def iden(g, b):
        return bool(np.all(np.asarray(g) == 1.0) and
                    np.all(np.asarray(b) == 0.0))
    return (iden(wd["ln1_g"], wd["ln1_b"]),
            iden(wd["lna_g"], wd["lna_b"]),
            iden(wd["lno_g"], wd["lno_b"]))


def kernel(**inputs):
    xs = {k: np.asarray(v) for k, v in inputs.items()}
    feat = np.ascontiguousarray(xs["feat"], dtype=np.float32)
    info = np.ascontiguousarray(xs["info"], dtype=np.float32)
    offsets = np.asarray(xs["offsets"]).astype(np.int64)
    n, c = feat.shape
    B = offsets.shape[0]
    assert c == NCORES * P

    bounds = np.concatenate([[0], offsets]).astype(np.int64)
    cw = 4096
    gb_id = _gb_identity(xs)
    nc = _get_program(tuple(int(v) for v in bounds), n, cw, gb_id)
    in_maps = _make_in_maps(feat, info, offsets, xs, B)

    if TRACE:
        _ensure_ntff_hook()
        import concourse.bass_utils as _bu
        _bu.upload_artifacts = lambda d: d  # no S3 in this container
    res = run_bass_kernel_spmd(nc, in_maps, core_ids=list(range(NCORES)),
                               trace=TRACE)
    global LAST_EXEC_NS, LAST_RESULTS
    LAST_EXEC_NS = res.exec_time_ns
    LAST_RESULTS = res
    featuresT = np.concatenate(
        [res.results[d]["featT_out"] for d in range(NCORES)], axis=0)  # (C, B)
    features = np.ascontiguousarray(featuresT.T)
    pred = res.results[0]["pred_out"]
    offset = res.results[0]["off_out"].reshape(B, 6, 3)
    return features, pred, offset
